# revision 25
# baseline (speedup 1.0000x reference)
"""TRN2 Bass kernel for nn_BatchDenseGAT (2-layer dense GAT, bs=32, n=512).

Sharding: data-parallel over the 32 graphs -> 4 graphs per NeuronCore x 8
cores, params replicated. Host does embedding gather/concat/transpose and
mask packing; all model math runs on device.

Device strategy per graph:
  exp(leaky_relu(s_i + d_j)) is factorized: with r_i = exp(-0.8*s_i),
  ed_j = exp(d_j), ed2_j = exp(0.2*d_j), the softmax-equivalent
  (row-normalization cancels any per-i factor) masked weight is
    em[j,i] = adjT[j,i] * max(r_i*ed2_j, ed_j)
  built with ONE dual-op tensor_scalar per [128,512] tile (DVE 4x mode)
  plus one bf16 mask multiply per head (split DVE/GpSimd) -- this removes
  the per-element Prelu+Exp work from the scalar engine; only one exp per
  head ([128,512] r_bc from the PE-replicated s broadcast) remains on ACT.
  out1T per head is [65,512] with a ones-column folded into the matmul
  lhsT giving the softmax denominator in row 64; 1/den comes from a DVE
  reciprocal of that PSUM row and is partition-broadcast by a contract-1
  PE outer product (ones column x rden row). elu uses
  elu(v) = min(exp(v),1) + max(v,0) - 1 ("-1" folded into layer-2 weights
  via negcs) with the min/add fused in one scalar_tensor_tensor.
  Layer 2 repeats the same factorized attention with one head and ends in
  a free-dim log_softmax.

  The graph loop is software-pipelined: FRONT(g+1) (loads, h_prime,
  tanh, d/s projections, attention-weight build) is issued before BACK(g)
  (aggregation matmuls, normalize, elu, layer 2) so each engine's
  in-order stream always has ready work while BACK(g)'s cross-engine
  chains drain.
"""
import os
import sys
import numpy as np

sys.path.insert(0, '/opt/trn_rl_repo')

import ml_dtypes  # noqa: E402
import concourse.bacc as bacc  # noqa: E402
import concourse.bass as bass  # noqa: E402
import concourse.tile as tile  # noqa: E402
from concourse import mybir  # noqa: E402

F32 = mybir.dt.float32
BF16 = mybir.dt.bfloat16
AF = mybir.ActivationFunctionType
ALU = mybir.AluOpType
BFNP = ml_dtypes.bfloat16

BS, N, NH, FO = 32, 512, 8, 64
FIN1 = 160
NCORES = 8
GPC = BS // NCORES  # graphs per core
NCH = 4             # 512 / 128 partition chunks
# heads whose adjacency-mask multiply runs on GpSimd instead of DVE
POOL_EM_HEADS = int(os.environ.get("GAT_POOL_EM", "5"))
POOL_EM2 = os.environ.get("GAT_POOL_EM2", "0") == "1"


def build_nc():
    B = 1 if os.environ.get("GAT_SERIAL", "0") == "1" else None
    nc = bacc.Bacc("TRN2", target_bir_lowering=False, debug=False)

    hT4 = nc.dram_tensor("hT4", [GPC, FIN1, N], BF16, kind="ExternalInput")
    adjTp = nc.dram_tensor("adjTp", [GPC, 128, NCH * N], BF16,
                           kind="ExternalInput")
    w1f_d = nc.dram_tensor("w1f", [FIN1, 512], BF16, kind="ExternalInput")
    # asrep[h] = a_src column for head h (rows of chunk h//2) replicated to
    # 128 columns; the other 3 oc chunks are all-zero so only oc=h//2 is kept
    asrep_d = nc.dram_tensor("asrep", [NH, 128, 128], BF16,
                             kind="ExternalInput")
    adst_d = nc.dram_tensor("adst", [512, NH], BF16, kind="ExternalInput")
    w2f_d = nc.dram_tensor("w2f", [512, 16], BF16, kind="ExternalInput")
    negcs_d = nc.dram_tensor("negcs", [1, 16], BF16, kind="ExternalInput")
    a2srep_d = nc.dram_tensor("a2srep", [16, 128], BF16, kind="ExternalInput")
    a2d_d = nc.dram_tensor("a2d", [16, 1], BF16, kind="ExternalInput")
    out_d = nc.dram_tensor("out", [GPC, N, 16], F32, kind="ExternalOutput")

    with tile.TileContext(nc) as tc:
        with tc.tile_pool(name="consts", bufs=1) as consts, \
             tc.tile_pool(name="gbuf", bufs=(B or 3)) as gbuf, \
             tc.tile_pool(name="attn", bufs=(B or 8)) as attn, \
             tc.tile_pool(name="emp", bufs=(B or 17)) as emp, \
             tc.tile_pool(name="small", bufs=(B or 2)) as small, \
             tc.tile_pool(name="elu", bufs=(B or 4)) as elup, \
             tc.tile_pool(name="fin", bufs=1) as finp, \
             tc.tile_pool(name="ps_big", bufs=(B or 3), space="PSUM") as ps_big, \
             tc.tile_pool(name="ps_rb", bufs=1, space="PSUM") as ps_rb, \
             tc.tile_pool(name="ps_sm", bufs=1, space="PSUM") as ps_sm, \
             tc.tile_pool(name="ps_sbc", bufs=(B or 3), space="PSUM") as ps_sbc:

            # ---------- constants ----------
            w1f_a = consts.tile([128, 512], BF16)
            w1f_b = consts.tile([32, 512], BF16)
            nc.sync.dma_start(out=w1f_a, in_=w1f_d.ap()[0:128, :])
            nc.sync.dma_start(out=w1f_b, in_=w1f_d.ap()[128:160, :])
            # (remaining consts are DMA'd after graph 0's loads -- see
            # load_consts() below -- so the first matmuls aren't stuck
            # behind the whole constant preamble in the DMA queue)
            asrep_sb = consts.tile([128, NH, 128], BF16)
            adst_sb = consts.tile([128, NCH, NH], BF16)
            w2f_sb = consts.tile([128, NCH, 16], BF16)
            negcs_sb = consts.tile([1, 16], BF16)
            a2srep_sb = consts.tile([16, 128], BF16)
            a2d_sb = consts.tile([16, 1], BF16)
            onesrow = consts.tile([1, 512], BF16)
            ones64 = consts.tile([1, 64], BF16)

            # adst is consumed by pre(0) (d projections) so it must be
            # DMA'd before pre(0) is issued; everything else can wait.
            nc.sync.dma_start(
                out=adst_sb,
                in_=adst_d.ap().rearrange("(c p) a -> p c a", c=NCH))

            def load_consts():
                nc.gpsimd.dma_start(
                    out=asrep_sb,
                    in_=asrep_d.ap().rearrange("a p q -> p a q"))
                nc.sync.dma_start(
                    out=w2f_sb,
                    in_=w2f_d.ap().rearrange("(c p) a -> p c a", c=NCH))
                nc.sync.dma_start(out=negcs_sb, in_=negcs_d.ap())
                nc.sync.dma_start(out=a2srep_sb, in_=a2srep_d.ap())
                nc.sync.dma_start(out=a2d_sb, in_=a2d_d.ap())
                nc.vector.memset(onesrow, 1.0)
                nc.vector.memset(ones64, 1.0)

            logits_all = finp.tile([128, GPC, NCH, 16], F32)
            S_all = finp.tile([128, GPC * NCH], F32)

            def prefetch(g):
                """Input DMAs for graph g (issued one cycle ahead)."""
                hT_a = gbuf.tile([128, N], BF16, tag="hT_a")
                hT_b = gbuf.tile([32, N], BF16, tag="hT_b")
                nc.scalar.dma_start(out=hT_a, in_=hT4.ap()[g, 0:128, :])
                nc.scalar.dma_start(out=hT_b, in_=hT4.ap()[g, 128:160, :])
                adjT_sb = gbuf.tile([128, NCH * N], BF16, tag="adjT")
                nc.sync.dma_start(out=adjT_sb, in_=adjTp.ap()[g])
                return dict(hT_a=hT_a, hT_b=hT_b, adjT=adjT_sb)

            def pre(pf):
                """h_prime both layouts, tanh, d projections and exp'd
                per-j scalars for a prefetched graph."""
                hT_a, hT_b, adjT_sb = pf["hT_a"], pf["hT_b"], pf["adjT"]

                def do_hpx():
                    # h_prime (n-major) -> hpx bf16 [i, (ic), (h, 64+ones)]
                    hpx = gbuf.tile([128, NCH, NH, 65], BF16, tag="hpx")
                    nc.vector.memset(hpx[:, :, :, 64:65], 1.0)
                    for ic in range(NCH):
                        hp_ps = ps_sbc.tile([128, 512], F32, tag="sbcps")
                        nc.tensor.matmul(hp_ps[:],
                                         hT_a[:, ic * 128:(ic + 1) * 128],
                                         w1f_a[:], start=True, stop=False)
                        nc.tensor.matmul(hp_ps[:],
                                         hT_b[:, ic * 128:(ic + 1) * 128],
                                         w1f_b[:], start=False, stop=True)
                        nc.scalar.activation(
                            hpx[:, ic, :, 0:64],
                            hp_ps[:].rearrange("p (h o) -> p h o", h=NH),
                            AF.Copy)
                    return hpx

                # tanh goes first: the whole s/d/attention chain of THIS
                # cycle hangs off tT, while hpx is only consumed by the NEXT
                # cycle's aggregation.
                hpx = None

                # h_primeT (o-major) -> tanh -> tT bf16
                tT = gbuf.tile([128, NCH, 512], BF16, tag="tT")
                for oc in range(NCH):
                    hpT_ps = ps_sbc.tile([128, 512], F32, tag="sbcps")
                    nc.tensor.matmul(hpT_ps[:],
                                     w1f_a[:, oc * 128:(oc + 1) * 128],
                                     hT_a[:], start=True, stop=False)
                    nc.tensor.matmul(hpT_ps[:],
                                     w1f_b[:, oc * 128:(oc + 1) * 128],
                                     hT_b[:], start=False, stop=True)
                    nc.scalar.activation(tT[:, oc, :], hpT_ps[:], AF.Tanh)

                # d cols + exp'd per-j scalars
                d_sb = small.tile([128, NCH, NH], F32, tag="d_sb")
                for jc in range(NCH):
                    d_ps = ps_sbc.tile([128, 512], F32, tag="sbcps")
                    for oc in range(NCH):
                        nc.tensor.matmul(d_ps[:, 0:NH],
                                         tT[:, oc, jc * 128:(jc + 1) * 128],
                                         adst_sb[:, oc, :],
                                         start=(oc == 0), stop=(oc == NCH - 1))
                    nc.vector.tensor_copy(d_sb[:, jc, :], d_ps[:, 0:NH])
                ed_sb = small.tile([128, NCH, NH], F32, tag="ed")
                nc.scalar.activation(ed_sb[:], d_sb[:], AF.Exp)
                ed2_sb = small.tile([128, NCH, NH], F32, tag="ed2")
                nc.scalar.activation(ed2_sb[:], d_sb[:], AF.Exp, scale=0.2)

                if hpx is None:
                    hpx = do_hpx()
                h2cT = gbuf.tile([128, NCH, 512], BF16, tag="h2cT")
                return dict(adjT=adjT_sb, hpx=hpx, tT=tT, ed=ed_sb,
                            ed2=ed2_sb, em=[], h2cT=h2cT, pair={})

            def attn_head(st, g, h):
                """Masked attention weights for one head of graph g."""
                # s broadcast from PE: each lhsT column is the same a_src
                # chunk, so the matmul emits s replicated across partitions.
                # Only chunk oc=h//2 of the padded a_src is nonzero, so one
                # matmul (contract 128) suffices.
                s_bc = ps_sbc.tile([128, 512], F32, tag="sbcps")
                nc.tensor.matmul(s_bc[:], asrep_sb[:, h, :],
                                 st["tT"][:, h // 2, :],
                                 start=True, stop=True)
                r_bc = attn.tile([128, 512], BF16, tag="rbc")
                nc.scalar.activation(r_bc[:], s_bc[:], AF.Exp, scale=-0.8)
                z_all = attn.tile([128, NCH * 512], BF16, tag="z")
                for jc in range(NCH):
                    nc.vector.tensor_scalar(
                        z_all[:, jc * 512:(jc + 1) * 512], r_bc[:],
                        st["ed2"][:, jc, h:h + 1], st["ed"][:, jc, h:h + 1],
                        op0=ALU.mult, op1=ALU.max)
                em_all = emp.tile([128, NCH * 512], BF16, tag="em")
                if h < POOL_EM_HEADS:
                    nc.gpsimd.tensor_mul(em_all[:], z_all[:], st["adjT"][:])
                else:
                    nc.vector.tensor_mul(em_all[:], z_all[:], st["adjT"][:])
                st["em"].append(em_all)

            def rb_and_v(h, o65_sb, rden_t, vp):
                # rden row (bf16) -> [64,512] PSUM via a contract-1 PE outer
                # product against a ones row; v = out1 * (1/den) on DVE
                # reads the ACT-evacuated out1 (SBUF) + rb (the one allowed
                # PSUM operand).
                rb_ps = ps_rb.tile([64, 512], F32, tag="rbps")
                nc.tensor.matmul(rb_ps[:], ones64[:], rden_t[:],
                                 start=True, stop=True)
                nc.vector.tensor_mul(vp[(h % 2) * 64:(h % 2) * 64 + 64, :],
                                     o65_sb[0:64, :], rb_ps[:])

            def agg_head(st, g, h):
                """Aggregation matmuls, normalize, elu for one head."""
                em_all = st["em"][h]
                hpx, h2cT, pair = st["hpx"], st["h2cT"], st["pair"]
                o65_ps = ps_big.tile([128, 512], F32, tag="bigps")
                for jc in range(NCH):
                    nc.tensor.matmul(
                        o65_ps[0:65, :], hpx[:, jc, h, :],
                        em_all[:, jc * 512:(jc + 1) * 512],
                        start=(jc == 0), stop=(jc == NCH - 1))
                rden_t = elup.tile([1, 512], BF16, tag="rden")
                with nc.allow_low_precision(reason="softmax denom recip"):
                    nc.vector.reciprocal(rden_t[:], o65_ps[64:65, :])
                o65_sb = elup.tile([64, 512], BF16, tag="o65sb")
                nc.scalar.activation(o65_sb[:], o65_ps[0:64, :], AF.Copy)
                if h % 2 == 0:
                    v_new = elup.tile([128, 512], BF16, tag="v")
                    pair["v"] = v_new
                    pair["rden"], pair["o65"] = rden_t, o65_sb
                    return
                v_pair = pair["v"]
                rb_and_v(h - 1, pair["o65"], pair["rden"], v_pair)
                rb_and_v(h, o65_sb, rden_t, v_pair)
                # elu(v) = min(exp(v),1) + max(v,0) - 1 (the -1 lives in
                # negcs folded into layer-2 weights)
                ev = elup.tile([128, 512], BF16, tag="ev")
                nc.scalar.activation(ev[:], v_pair[:], AF.Exp)
                p_pair = elup.tile([128, 512], BF16, tag="p")
                nc.vector.tensor_scalar_max(p_pair[:], v_pair[:], 0.0)
                nc.vector.scalar_tensor_tensor(h2cT[:, h // 2, :], ev[:],
                                               1.0, p_pair[:],
                                               op0=ALU.min, op1=ALU.add)

            def post_stages(st, g):
                """Layer 2 + per-graph softmax for graph g, split into 8
                stages issued one per head slot so the serial chain's
                latency hides under attn/agg engine work."""
                adjT_sb, h2cT = st["adjT"], st["h2cT"]
                ps = {}

                def s0():
                    # h_prime2 (n-major) [i, 16] + ones col -> hp2x bf16
                    hp2x = small.tile([128, NCH, 17], BF16, tag="hp2x")
                    ps["hp2x"] = hp2x
                    nc.vector.memset(hp2x[:, :, 16:17], 1.0)
                    for ic in range(2):
                        hp2_ps = ps_sm.tile([128, 17], F32, tag="smps")
                        for fc in range(NCH):
                            nc.tensor.matmul(
                                hp2_ps[:, 0:16],
                                h2cT[:, fc, ic * 128:(ic + 1) * 128],
                                w2f_sb[:, fc, :],
                                start=(fc == 0), stop=False)
                        nc.tensor.matmul(hp2_ps[:, 0:16],
                                         onesrow[:, ic * 128:(ic + 1) * 128],
                                         negcs_sb[:], start=False, stop=True)
                        nc.vector.tensor_copy(hp2x[:, ic, 0:16],
                                              hp2_ps[:, 0:16])

                def s1():
                    hp2x = ps["hp2x"]
                    for ic in range(2, NCH):
                        hp2_ps = ps_sm.tile([128, 17], F32, tag="smps")
                        for fc in range(NCH):
                            nc.tensor.matmul(
                                hp2_ps[:, 0:16],
                                h2cT[:, fc, ic * 128:(ic + 1) * 128],
                                w2f_sb[:, fc, :],
                                start=(fc == 0), stop=False)
                        nc.tensor.matmul(hp2_ps[:, 0:16],
                                         onesrow[:, ic * 128:(ic + 1) * 128],
                                         negcs_sb[:], start=False, stop=True)
                        nc.vector.tensor_copy(hp2x[:, ic, 0:16],
                                              hp2_ps[:, 0:16])
                    # h_prime2T [16, n] -> tanh -> t2 bf16
                    hp2T_ps = ps_sbc.tile([128, 512], F32, tag="sbcps")
                    for fc in range(NCH):
                        nc.tensor.matmul(hp2T_ps[0:16, :], w2f_sb[:, fc, :],
                                         h2cT[:, fc, :],
                                         start=(fc == 0), stop=False)
                    nc.tensor.matmul(hp2T_ps[0:16, :], negcs_sb[:],
                                     onesrow[:], start=False, stop=True)
                    t2_sb = small.tile([16, 512], BF16, tag="t2")
                    ps["t2"] = t2_sb
                    nc.scalar.activation(t2_sb[:], hp2T_ps[0:16, :], AF.Tanh)

                def s2():
                    t2_sb = ps["t2"]
                    # s2 broadcast via replicated-column matmul; d2 cols
                    s2_ps = ps_sbc.tile([128, 512], F32, tag="sbcps")
                    nc.tensor.matmul(s2_ps[:], a2srep_sb[:], t2_sb[:],
                                     start=True, stop=True)
                    r2_bc = attn.tile([128, 512], BF16, tag="rbc")
                    ps["r2"] = r2_bc
                    nc.scalar.activation(r2_bc[:], s2_ps[:], AF.Exp,
                                         scale=-0.8)
                    d2_sb = small.tile([128, NCH], F32, tag="d2")
                    ps["d2"] = d2_sb
                    for jc in range(NCH):
                        d2_ps = ps_sm.tile([128, 17], F32, tag="smps")
                        nc.tensor.matmul(d2_ps[:, 0:1],
                                         t2_sb[:, jc * 128:(jc + 1) * 128],
                                         a2d_sb[:], start=True, stop=True)
                        nc.vector.tensor_copy(d2_sb[:, jc:jc + 1],
                                              d2_ps[:, 0:1])

                def s3():
                    d2_sb = ps["d2"]
                    e2d_sb = small.tile([128, NCH], F32, tag="e2d")
                    nc.scalar.activation(e2d_sb[:], d2_sb[:], AF.Exp)
                    e2d2_sb = small.tile([128, NCH], F32, tag="e2d2")
                    nc.scalar.activation(e2d2_sb[:], d2_sb[:], AF.Exp,
                                         scale=0.2)
                    z2_all = attn.tile([128, NCH * 512], BF16, tag="z")
                    ps["z2"] = z2_all
                    for jc in range(NCH):
                        nc.vector.tensor_scalar(
                            z2_all[:, jc * 512:(jc + 1) * 512], ps["r2"][:],
                            e2d2_sb[:, jc:jc + 1], e2d_sb[:, jc:jc + 1],
                            op0=ALU.mult, op1=ALU.max)

                def s4():
                    em2_all = emp.tile([128, NCH * 512], BF16, tag="em")
                    ps["em2"] = em2_all
                    if POOL_EM2:
                        nc.gpsimd.tensor_mul(em2_all[:], ps["z2"][:],
                                             adjT_sb[:])
                    else:
                        nc.vector.tensor_mul(em2_all[:], ps["z2"][:],
                                             adjT_sb[:])

                def out2_ic(ic):
                    o2_ps = ps_sm.tile([128, 17], F32, tag="smps")
                    for jc in range(NCH):
                        nc.tensor.matmul(
                            o2_ps[:],
                            ps["em2"][:, jc * 512 + ic * 128:
                                      jc * 512 + (ic + 1) * 128],
                            ps["hp2x"][:, jc, :],
                            start=(jc == 0), stop=(jc == NCH - 1))
                    r2c = small.tile([128, 1], F32, tag="r2c")
                    nc.vector.reciprocal(r2c[:], o2_ps[:, 16:17])
                    nc.vector.tensor_scalar_mul(logits_all[:, g, ic, :],
                                                o2_ps[:, 0:16], r2c[:])
                    ex = small.tile([128, 16], F32, tag="ex")
                    nc.scalar.activation(ex[:], logits_all[:, g, ic, :],
                                         AF.Exp,
                                         accum_out=S_all[:, g * NCH + ic:
                                                         g * NCH + ic + 1])

                def s5():
                    out2_ic(0)
                    out2_ic(1)

                def s6():
                    out2_ic(2)
                    out2_ic(3)

                return [s0, s1, s2, s3, s4, s5, s6]

            def finale(g0, ng):
                """Deferred log-softmax completion for graphs g0..g0+ng-1.
                Ln is the only activation outside the Exp/Tanh/Copy table,
                so batching it across graphs avoids per-graph table swaps."""
                L_g = small.tile([128, ng * NCH], F32, tag=f"L_g{ng}")
                nc.scalar.activation(L_g[:],
                                     S_all[:, g0 * NCH:(g0 + ng) * NCH],
                                     AF.Ln)
                for gi in range(ng):
                    for ic in range(NCH):
                        g = g0 + gi
                        fin = small.tile([128, 16], F32, tag="fin")
                        nc.vector.tensor_scalar_sub(
                            fin[:], logits_all[:, g, ic, :],
                            L_g[:, gi * NCH + ic:gi * NCH + ic + 1])
                        nc.sync.dma_start(
                            out=out_d.ap()[g, ic * 128:(ic + 1) * 128, :],
                            in_=fin[:])

            # ------- head-interleaved software-pipelined graph loop -------
            # cycle c issues: pre(c), then per-head slots pairing the
            # attention-weight build of graph c with the aggregation/elu of
            # graph c-1 and one layer-2 stage of graph c-2. Input DMAs are
            # prefetched one cycle ahead; the Ln+output finale is batched.
            states = {}
            pfs = {0: prefetch(0), 1: prefetch(1)}
            states[0] = pre(pfs.pop(0))
            load_consts()
            for h in range(NH):
                attn_head(states[0], 0, h)
            for c in range(1, GPC + 1):
                if c < GPC:
                    states[c] = pre(pfs.pop(c))
                    if c + 1 < GPC:
                        pfs[c + 1] = prefetch(c + 1)
                stages = (post_stages(states.pop(c - 2), c - 2)
                          if c >= 2 else [])
                for h in range(NH):
                    # agg first: its inputs (graph c-1) are all ready, so
                    # engine queue heads never block on graph c's pre chain
                    agg_head(states[c - 1], c - 1, h)
                    if c < GPC:
                        attn_head(states[c], c, h)
                    if h < len(stages):
                        stages[h]()
            # batched Ln + outputs for all but the last graph; the last
            # graph's layer 2 drains behind it
            finale(0, GPC - 1)
            for sfn in post_stages(states.pop(GPC - 1), GPC - 1):
                sfn()
            finale(GPC - 1, 1)

    return nc


def host_prep(adj, vertices, local_emb, emb0, emb1, w1, a_src1, a_dst1,
              w2, a_src2, a_dst2):
    """Build the 8 per-core input maps from full inputs."""
    adj = np.asarray(adj, dtype=np.float32)
    vertices = np.asarray(vertices)
    local_emb = np.asarray(local_emb, dtype=np.float32)
    emb0 = np.asarray(emb0, dtype=np.float32)
    emb1 = np.asarray(emb1, dtype=np.float32)
    w1 = np.asarray(w1, dtype=np.float32)
    a_src1 = np.asarray(a_src1, dtype=np.float32)
    a_dst1 = np.asarray(a_dst1, dtype=np.float32)
    w2 = np.asarray(w2, dtype=np.float32)
    a_src2 = np.asarray(a_src2, dtype=np.float32)
    a_dst2 = np.asarray(a_dst2, dtype=np.float32)

    vtx = vertices.astype(np.int64)
    # h: [b, n, 160] -> hT [b, 160, n]
    h = np.concatenate([emb0[vtx], emb1[vtx], local_emb], axis=2)
    hT = np.ascontiguousarray(h.transpose(0, 2, 1)).astype(BFNP)

    # adjT packed: [b, 128, 4*512] bf16, block jc = adjT rows jc*128..
    adjT = adj.transpose(0, 2, 1)
    adjTp = np.ascontiguousarray(
        adjT.reshape(BS, NCH, 128, N).transpose(0, 2, 1, 3).reshape(
            BS, 128, NCH * N)).astype(BFNP)

    w1f = np.ascontiguousarray(w1.transpose(1, 0, 2).reshape(FIN1, 512))
    asrc = np.zeros((512, NH), np.float32)
    adst = np.zeros((512, NH), np.float32)
    for hh in range(NH):
        asrc[hh * 64:(hh + 1) * 64, hh] = a_src1[hh, :, 0]
        adst[hh * 64:(hh + 1) * 64, hh] = a_dst1[hh, :, 0]
    asrep = np.zeros((NH, 128, 128), np.float32)
    for hh in range(NH):
        oc = hh // 2
        asrep[hh] = asrc[oc * 128:(oc + 1) * 128, hh:hh + 1]
    a2srep = np.repeat(a_src2[0], 128, axis=1)  # [16, 128]
    consts = {
        "w1f": w1f.astype(BFNP),
        "asrep": asrep.astype(BFNP),
        "adst": adst.astype(BFNP),
        "w2f": w2[0].astype(BFNP),
        "negcs": (-w2[0].sum(axis=0, keepdims=True)).astype(BFNP),
        "a2srep": a2srep.astype(BFNP),
        "a2d": a_dst2[0].astype(BFNP),
    }
    in_maps = []
    for core in range(NCORES):
        sl = slice(core * GPC, (core + 1) * GPC)
        m = dict(consts)
        m["hT4"] = np.ascontiguousarray(hT[sl])
        m["adjTp"] = np.ascontiguousarray(adjTp[sl])
        in_maps.append(m)
    return in_maps


_NC_CACHE = {}


def _get_nc():
    if "nc" not in _NC_CACHE:
        nc = build_nc()
        nc.compile()
        _NC_CACHE["nc"] = nc
    return _NC_CACHE["nc"]


def kernel(**inputs):
    from concourse.bass_utils import run_bass_kernel_spmd
    nc = _get_nc()
    in_maps = host_prep(**inputs)
    res = run_bass_kernel_spmd(nc, in_maps, core_ids=list(range(NCORES)))
    out = np.concatenate([r["out"] for r in res.results], axis=0)
    return out.astype(np.float32)


if __name__ == "__main__":
    nc = build_nc()
    print("built ok")

